# revision 45
# baseline (speedup 1.0000x reference)
"""GQA attention kernel for Trainium2 (Bass/Tile), 8-core SPMD. v4.

Problem: B=2, N=2048, DIM=1024, 16 query heads / 4 KV heads, head_dim=64, fp32.
Sharding: core c = (batch b=c//4, kv-group g=c%4): each core computes its
group's 4 query heads + 1 shared KV head over the full sequence and a partial
output projection (its 256 rows of Wo). Host sums the 4 group partials per
batch and adds the bias.

Design (cost-model-driven; Act-engine exp of the 16.8M scores is the floor):
  - Host passes x^T per batch in bf16: no PE transposes of x, half the DMA.
  - Matmuls orient so the MOVING operand is small (PE cost = moving cols):
    P@V uses P^T tiles as stationary and V[128,65] as moving (131k -> 66.5k
    cycles) and lands as [q-part, d], making softmax normalization a
    per-partition scalar multiply (Pool engine).
  - Scores S^T stream per step gqc=(qchunk, head); exp in [128,3,512] PSUM
    tiles (two 3-bank buffers); P^T in a 3-slot SBUF ring; PV lags 1 step.
  - All non-score PE work (PV blocks, projections, output projection) sits in
    a cost-budgeted filler queue drained between score tiles so the in-order
    PE stream never starves the Act engine.
"""

import sys

if "/opt/trn_rl_repo" not in sys.path:
    sys.path.insert(0, "/opt/trn_rl_repo")

from collections import deque
from contextlib import ExitStack

import numpy as np

import concourse.bass as bass
import concourse.mybir as mybir
import concourse.tile as tile
from concourse import bacc, bass_utils
from concourse.bass import ds, ts
from concourse.masks import make_identity

F32 = mybir.dt.float32
BF16 = mybir.dt.bfloat16
EXPF = mybir.ActivationFunctionType.Exp

DIM = 1024
D = 64          # head dim
H = 4           # query heads per core
SCALE = D ** -0.5
N_WARM = 30     # PE clock-ramp dummy transposes (~4.5us, spans the input DMA)
GAP_NS = 800    # filler budget per exp gap


def build_nc(NSEQ=2048):
    KT = NSEQ // 128    # key tiles
    QC = NSEQ // 512    # query chunks
    DKT = DIM // 128    # contraction chunks for projections
    NG = H * QC         # steps: gqc = qc*H + h
    NT = (KT + 2) // 3  # scores/exp tiles per step

    nc = bacc.Bacc("TRN2", target_bir_lowering=False, debug=False)
    # weights arrive pre-arranged by the host into their SBUF layout so each
    # DMA partition line is one long contiguous run (<512B runs pay 2x DMA);
    # wq is head-pair-major so the p=0 half can be DMA'd first
    xT = nc.dram_tensor("xt", [DIM, NSEQ], BF16, kind="ExternalInput").ap()
    wq = nc.dram_tensor("wq", [128, 2 * DKT * 128], BF16, kind="ExternalInput").ap()
    wkk = nc.dram_tensor("wkk", [128, DKT * 128], BF16, kind="ExternalInput").ap()
    wv = nc.dram_tensor("wv", [128, DKT * D], BF16, kind="ExternalInput").ap()
    wo = nc.dram_tensor("wo", [128, 2 * DIM], BF16, kind="ExternalInput").ap()
    out = nc.dram_tensor("out", [DIM, NSEQ], BF16, kind="ExternalOutput").ap()

    with tile.TileContext(nc) as tc, ExitStack() as ctx:
        sb = ctx.enter_context(tc.tile_pool(name="sb", bufs=1))

        wq_sb = sb.tile([128, 2, DKT, 128], BF16)
        wkk_sb = sb.tile([128, DKT, 128], BF16)
        wv_sb = sb.tile([128, DKT, D], BF16)
        wo_sb = sb.tile([128, 2, DIM], BF16)
        identb = sb.tile([128, 128], BF16)
        ones_k = sb.tile([128, 1], BF16)
        warm = sb.tile([128, 1], F32)

        xts = sb.tile([128, DKT, NSEQ], BF16)     # x^T, d-chunk t at [:, t, :]
        kkT = sb.tile([128, NSEQ], BF16)          # K^T duplicated rows 0-63 == 64-127
        qt = [sb.tile([128, NSEQ], BF16, name=f"qt{p}") for p in range(2)]
        vn = sb.tile([128, KT, D + 1], BF16)      # V seq-major + ones col 64
        # P^T ring, flat over 48 units of [128,512]; unit (gqc,kt) lives at
        # ring index u%48 = (gqc%3)*KT + kt, so consecutive units are
        # contiguous and every exp tile is a full 3 units
        ptr = sb.tile([128, 3 * KT * 512], BF16)
        aout = sb.tile([128, H, NSEQ // 128, D], BF16)
        aoutT = sb.tile([128, 2, NSEQ], BF16)

        make_identity(nc, identb)
        nc.vector.memset(ones_k, 1.0)
        nc.vector.memset(vn, 1.0)
        # preload the exp table off the critical path
        nc.scalar.activation(out=warm, in_=ones_k, func=EXPF, scale=1.0)

        # input DMA: first x chunk and the K/Q weights lead the queue (they
        # gate pipeline start); V/O weights ride the gpsimd queue.
        def dma_xchunk(qc):
            nc.sync.dma_start(
                out=xts[:, :, ds(qc * 512, 512)],
                in_=xT[:, ds(qc * 512, 512)].rearrange("(t p) m -> p t m", p=128),
            )

        # single serial DMA device: strict priority order on one queue; the
        # first x chunk is split so K projection starts on its first half
        nc.sync.dma_start(out=wkk_sb, in_=wkk.rearrange("p (t m) -> p t m", t=DKT))
        for dh in range(2):
            nc.sync.dma_start(
                out=xts[:, ds(dh * 4, 4), 0:512],
                in_=xT[ds(dh * 512, 512), 0:512].rearrange("(t p) m -> p t m", p=128),
            )
        nc.sync.dma_start(out=wq_sb[:, 0, :, :],
                          in_=wq[:, 0:DKT * 128].rearrange("p (t m) -> p t m", t=DKT))
        # later chunks in column halves: kkT key-tiles become available at the
        # pace the Act engine consumes them in step 0
        for qc in range(1, QC):
            for hh in range(2):
                c0 = qc * 512 + hh * 256
                nc.sync.dma_start(
                    out=xts[:, :, ds(c0, 256)],
                    in_=xT[:, ds(c0, 256)].rearrange("(t p) m -> p t m", p=128),
                )
        nc.sync.dma_start(out=wq_sb[:, 1, :, :],
                          in_=wq[:, DKT * 128:].rearrange("p (t m) -> p t m", t=DKT))
        nc.sync.dma_start(out=wv_sb, in_=wv.rearrange("p (t m) -> p t m", t=DKT))
        nc.sync.dma_start(out=wo_sb, in_=wo.rearrange("p (t m) -> p t m", t=2))

        scp = ctx.enter_context(tc.tile_pool(name="scp", bufs=2, space="PSUM"))
        psp = ctx.enter_context(tc.tile_pool(name="psp", bufs=2, space="PSUM"))
        rp = ctx.enter_context(tc.tile_pool(name="rp", bufs=2))
        stp = ctx.enter_context(tc.tile_pool(name="stp", bufs=4))

        # PE clock warm-up: dependency-free transposes keep the tensor engine
        # busy through the input DMA so real work starts at full clock.
        wps = psp.tile([128, 1024], BF16, tag="ps", name="warmps")
        for _ in range(N_WARM):
            nc.tensor.transpose(wps[:, 0:128], identb, identb)

        def emit_k(qc, half=None):
            c0, w = (qc * 512, 512) if half is None else (qc * 512 + half * 256, 256)
            ps = psp.tile([128, 512], F32, tag="ps", name=f"kps{qc}_{half}")
            for d in range(DKT):
                nc.tensor.matmul(ps[:, 0:w], wkk_sb[:, d, :], xts[:, d, ds(c0, w)],
                                 start=(d == 0), stop=(d == DKT - 1))
            nc.vector.tensor_copy(kkT[:, ds(c0, w)], ps[:, 0:w])

        def emit_q(p, qc):
            ps = psp.tile([128, 512], F32, tag="ps", name=f"qps{p}_{qc}")
            for d in range(DKT):
                nc.tensor.matmul(ps, wq_sb[:, p, d, :],
                                 xts[:, d, ds(qc * 512, 512)],
                                 start=(d == 0), stop=(d == DKT - 1))
            nc.vector.tensor_copy(qt[p][:, ds(qc * 512, 512)], ps)

        def emit_v(st):
            ps = psp.tile([128, 512], F32, tag="ps", name=f"vps{st}")
            for d in range(DKT):
                nc.tensor.matmul(ps[:, 0:D], xts[:, d, ds(st * 128, 128)],
                                 wv_sb[:, d, :],
                                 start=(d == 0), stop=(d == DKT - 1))
            nc.vector.tensor_copy(vn[:, st, 0:D], ps[:, 0:D])

        ustate = {"u": 0, "psc": None}

        def emit_unit(gqc, kt):
            """One 512-col scores matmul; exp flushes every 3 units."""
            u = ustate["u"]
            qc, h = divmod(gqc, H)
            p, hb = h // 2, (h % 2) * 64
            pos = u % 3
            if pos == 0:
                ustate["psc"] = scp.tile([128, 1536], F32, tag="sc", name=f"sc{u}")
            nc.tensor.matmul(ustate["psc"][:, ds(pos * 512, 512)],
                             kkT[ds(hb, 64), ts(kt, 128)],
                             qt[p][ds(hb, 64), ds(qc * 512, 512)],
                             start=True, stop=True)
            ustate["u"] = u + 1
            if pos == 2 or u == NG * KT - 1:
                ul = pos + 1
                ru0 = (u - pos) % (3 * KT)
                nc.scalar.activation(out=ptr[:, ds(ru0 * 512, ul * 512)],
                                     in_=ustate["psc"][:, 0:ul * 512],
                                     func=EXPF, scale=SCALE)
                drain()
                base = (NG - 1) * KT
                if u == base + 5:
                    pv_partial(NG - 1, 0, 6)
                elif u == base + 8:
                    pv_partial(NG - 1, 6, 9)
                elif u == base + 14:
                    pv_partial(NG - 1, 9, 15)

        # ---- filler queue: (est PE ns, emit_fn), drained between score tiles
        F = deque()

        def drain(budget=GAP_NS):
            spent = 0
            while F and spent < budget:
                cost, fn = F.popleft()
                fn()
                spent += cost

        pv_state = {}

        def pv_partial(gqc, lo, hi):
            """PV accumulation for all 4 query subtiles over key tiles [lo,hi)."""
            slot = gqc % 3
            if lo == 0:
                pv_state[gqc] = psp.tile([128, 512], F32, tag="ps", name=f"pv{gqc}")
            pv = pv_state[gqc]
            for j in range(4):
                for kt in range(lo, hi):
                    # start only on the tile's very first matmul: a start on
                    # j>0 would re-mark the whole PSUM bank pending-zero and
                    # wipe j0's partial sums on its next accumulate
                    nc.tensor.matmul(pv[:, ds(j * 65, 65)],
                                     ptr[:, ds((slot * KT + kt) * 512 + j * 128, 128)],
                                     vn[:, kt, :],
                                     start=(kt == 0 and j == 0), stop=(kt == KT - 1),
                                     skip_group_check=True)

        def pv_norm(gqc):
            qc, h = divmod(gqc, H)
            pv = pv_state.pop(gqc)
            pvs = rp.tile([128, 4 * (D + 1)], F32, tag="pvs", name=f"pvs{gqc}")
            nc.vector.tensor_copy(pvs, pv[:, 0:4 * (D + 1)])
            r = rp.tile([128, 4], F32, tag="r", name=f"r{gqc}")
            for j in range(4):
                nc.vector.reciprocal(out=r[:, ds(j, 1)],
                                     in_=pvs[:, ds(j * 65 + D, 1)])
            for j in range(4):
                nc.gpsimd.tensor_scalar_mul(aout[:, h, qc * 4 + j, :],
                                            pvs[:, ds(j * 65, D)], r[:, ds(j, 1)])

        def q_pv(gqc):
            """Queue PV j-blocks + normalization for step gqc."""

            def mk_j(j):
                def go():
                    slot = gqc % 3
                    if j == 0:
                        pv_state[gqc] = psp.tile([128, 512], F32, tag="ps",
                                                 name=f"pv{gqc}")
                    pv = pv_state[gqc]
                    for kt in range(KT):
                        nc.tensor.matmul(pv[:, ds(j * 65, 65)],
                                         ptr[:, ds((slot * KT + kt) * 512 + j * 128, 128)],
                                         vn[:, kt, :],
                                         start=(kt == 0 and j == 0), stop=(kt == KT - 1),
                                         skip_group_check=True)
                return go

            for j in range(4):
                F.append((433, mk_j(j)))
            F.append((0, lambda: pv_norm(gqc)))

        def q_tr_chunk(qc, c):
            """Transpose head-pair c (heads 2c, 2c+1) of qchunk qc into aoutT."""
            box = {}

            def mk_tr(hh):
                def go():
                    if hh % 2 == 0:
                        box["t"] = psp.tile([128, 1024], BF16, tag="ps",
                                            name=f"tr{qc}_{c}")
                    tr = box["t"]
                    for j in range(4):
                        nc.tensor.transpose(
                            tr[ds((hh % 2) * 64, 64), ds(j * 128, 128)],
                            aout[:, hh, qc * 4 + j, :], identb)
                return go

            def tr_copy():
                nc.vector.tensor_copy(aoutT[:, c, ds(qc * 512, 512)],
                                      box.pop("t")[:, 0:512])

            for hh in (2 * c, 2 * c + 1):
                F.append((213, mk_tr(hh)))
            F.append((0, tr_copy))

        def q_ods(qc, lo, hi, tail=False):
            """Output projection rows [128*lo, 128*hi); ods are processed in
            pairs sharing one staging tile and one DMA."""
            stt_box = {}

            def mk_od(od, use_scp, act_copy):
                def go():
                    if use_scp:
                        op = scp.tile([128, 3, 512], F32, tag="sc",
                                      name=f"op{qc}_{od}")[:, 0, :]
                    else:
                        op = psp.tile([128, 512], F32, tag="ps", name=f"op{qc}_{od}")
                    nc.tensor.matmul(op, wo_sb[:, 0, ts(od, 128)],
                                     aoutT[:, 0, ds(qc * 512, 512)],
                                     start=True, stop=False)
                    nc.tensor.matmul(op, wo_sb[:, 1, ts(od, 128)],
                                     aoutT[:, 1, ds(qc * 512, 512)],
                                     start=False, stop=True)
                    if od % 2 == 0:
                        stt_box[od] = stp.tile([128, 2, 512], BF16, tag="st",
                                               name=f"st{qc}_{od}")
                    stt = stt_box.get(od, stt_box.get(od - 1))
                    dst = stt[:, od % 2, :]
                    if act_copy:
                        nc.scalar.activation(out=dst, in_=op,
                                             func=mybir.ActivationFunctionType.Copy,
                                             scale=1.0)
                    else:
                        nc.vector.tensor_copy(dst, op)
                    if tail:
                        eng = nc.gpsimd if od % 2 == 1 else nc.sync
                        eng.dma_start(out=out[ts(od, 128), ds(qc * 512, 512)],
                                      in_=dst)
                        if od % 2 == 1:
                            stt_box.pop(od - 1)
                    elif od % 2 == 1:
                        stt2 = stt_box.pop(od - 1)
                        nc.sync.dma_start(
                            out=out[ds((od - 1) * 128, 256), ds(qc * 512, 512)]
                            .rearrange("(c p) m -> p c m", p=128),
                            in_=stt2)
                return go

            for od in range(lo, hi):
                F.append((427, mk_od(od, tail and od % 2 == 0, tail and od % 2 == 1)))

        # ---- step 0: fixed interleave to bootstrap K/Q ----
        emit_k(0)
        emit_q(0, 0)
        for kt in range(KT):
            emit_unit(0, kt)
            if kt in (2, 5, 8):
                emit_k(kt // 3 + 1, 0)
                emit_k(kt // 3 + 1, 1)
        emit_q(1, 0)

        # ---- main loop ----
        # per-loop filler plan (each transpose group lands >=1 loop after the
        # normalization it reads, so PE never stalls on the norm chain):
        #   ph==3: tr-chunk0(qc4) + Q(0, qc4+1)
        #   ph==0: Q(1, qc4)
        #   ph==1: tr-chunk1(qc4-1) + output rows 0-3 of qc4-1
        #   ph==2: output rows 4-7 of qc4-1
        for gqc in range(1, NG):
            if gqc == 1:
                for st in range(KT):
                    F.append((213, (lambda s: lambda: emit_v(s))(st)))
            q_pv(gqc - 1)
            qc4, ph = divmod(gqc, 4)
            if ph == 3:
                q_tr_chunk(qc4, 0)
                if qc4 + 1 < QC:
                    F.append((1707, (lambda q: lambda: emit_q(0, q))(qc4 + 1)))
            elif ph == 0:
                F.append((1707, (lambda q: lambda: emit_q(1, q))(qc4)))
            elif ph == 1 and qc4 >= 1:
                q_tr_chunk(qc4 - 1, 1)
                q_ods(qc4 - 1, 0, 4)
            elif ph == 2 and qc4 >= 1:
                q_ods(qc4 - 1, 4, 8)
            for kt in range(KT):
                emit_unit(gqc, kt)
        # ---- tail: j-granular PV residue -> norm -> transpose chains so the
        # last step's latencies overlap; then the final output projection
        gq, qc = NG - 1, QC - 1
        slot = gq % 3
        trt = psp.tile([128, 1024], BF16, tag="ps", name="tr_tail")
        for j in range(4):
            nc.tensor.transpose(trt[ds(0, 64), ds(j * 128, 128)],
                                aout[:, 2, qc * 4 + j, :], identb)
        pv = pv_state.pop(gq)
        r = rp.tile([128, 4], F32, tag="r", name="r_tail")
        for j in range(4):
            nc.tensor.matmul(pv[:, ds(j * 65, 65)],
                             ptr[:, ds((slot * KT + KT - 1) * 512 + j * 128, 128)],
                             vn[:, KT - 1, :],
                             start=False, stop=True,
                             skip_group_check=True)
            nc.vector.reciprocal(out=r[:, ds(j, 1)], in_=pv[:, ds(j * 65 + D, 1)])
            nc.vector.tensor_scalar_mul(aout[:, 3, qc * 4 + j, :],
                                        pv[:, ds(j * 65, D)], r[:, ds(j, 1)])
        for j in range(4):
            nc.tensor.transpose(trt[ds(64, 64), ds(j * 128, 128)],
                                aout[:, 3, qc * 4 + j, :], identb)
        nc.vector.tensor_copy(aoutT[:, 1, ds(qc * 512, 512)], trt[:, 0:512])
        q_ods(qc, 0, 8, tail=True)
        drain(budget=1 << 30)

    nc.compile()
    return nc


_CACHE = {}


def _get_nc(NSEQ):
    if NSEQ not in _CACHE:
        _CACHE[NSEQ] = build_nc(NSEQ)
    return _CACHE[NSEQ]


def kernel(x, Wq, Wk, Wv, Wo, bo):
    """Full-input entry point: shard over 8 cores, run, gather."""
    import ml_dtypes

    bf16 = ml_dtypes.bfloat16
    x, Wq, Wk, Wv, Wo, bo = (np.asarray(a, np.float32) for a in (x, Wq, Wk, Wv, Wo, bo))
    B, N, C = x.shape
    nc = _get_nc(N)
    xT_b = [np.ascontiguousarray(x[b].T).astype(bf16) for b in range(B)]

    def arr(w):
        """Pre-arrange [128*t, m] weight to SBUF layout [128, t*m]."""
        t = w.shape[0] // 128
        return np.ascontiguousarray(
            w.reshape(t, 128, -1).transpose(1, 0, 2).reshape(128, -1)).astype(bf16)

    def arr_q(w):
        """Pre-arrange Wq [128*t, 2*128] to head-pair-major [128, pp*t*m]."""
        t = w.shape[0] // 128
        return np.ascontiguousarray(
            w.reshape(t, 128, 2, 128).transpose(1, 2, 0, 3).reshape(128, -1)).astype(bf16)

    in_maps = []
    for c in range(8):
        b, g = c // 4, c % 4
        wk_g = Wk[:, g * D:(g + 1) * D]
        in_maps.append({
            "xt": xT_b[b],
            "wq": arr_q(Wq[:, g * 256:(g + 1) * 256]),
            "wkk": arr(np.concatenate([wk_g, wk_g], axis=1)),
            "wv": arr(Wv[:, g * D:(g + 1) * D]),
            "wo": arr(Wo[g * 256:(g + 1) * 256, :]),
        })
    res = bass_utils.run_bass_kernel_spmd(nc, in_maps, core_ids=list(range(8)))
    outs = [res.results[c]["out"] for c in range(8)]
    full = np.empty((B, N, C), np.float32)
    for b in range(B):
        acc = outs[4 * b].astype(np.float32)
        for g in range(1, 4):
            acc = acc + outs[4 * b + g].astype(np.float32)
        full[b] = acc.T + bo[None, :]
    return full


# revision 46
# speedup vs baseline: 1.0028x; 1.0028x over previous
"""GQA attention kernel for Trainium2 (Bass/Tile), 8-core SPMD. v4.

Problem: B=2, N=2048, DIM=1024, 16 query heads / 4 KV heads, head_dim=64, fp32.
Sharding: core c = (batch b=c//4, kv-group g=c%4): each core computes its
group's 4 query heads + 1 shared KV head over the full sequence and a partial
output projection (its 256 rows of Wo). Host sums the 4 group partials per
batch and adds the bias.

Design (cost-model-driven; Act-engine exp of the 16.8M scores is the floor):
  - Host passes x^T per batch in bf16: no PE transposes of x, half the DMA.
  - Matmuls orient so the MOVING operand is small (PE cost = moving cols):
    P@V uses P^T tiles as stationary and V[128,65] as moving (131k -> 66.5k
    cycles) and lands as [q-part, d], making softmax normalization a
    per-partition scalar multiply (Pool engine).
  - Scores S^T stream per step gqc=(qchunk, head); exp in [128,3,512] PSUM
    tiles (two 3-bank buffers); P^T in a 3-slot SBUF ring; PV lags 1 step.
  - All non-score PE work (PV blocks, projections, output projection) sits in
    a cost-budgeted filler queue drained between score tiles so the in-order
    PE stream never starves the Act engine.
"""

import sys

if "/opt/trn_rl_repo" not in sys.path:
    sys.path.insert(0, "/opt/trn_rl_repo")

from collections import deque
from contextlib import ExitStack

import numpy as np

import concourse.bass as bass
import concourse.mybir as mybir
import concourse.tile as tile
from concourse import bacc, bass_utils
from concourse.bass import ds, ts
from concourse.masks import make_identity

F32 = mybir.dt.float32
BF16 = mybir.dt.bfloat16
EXPF = mybir.ActivationFunctionType.Exp

DIM = 1024
D = 64          # head dim
H = 4           # query heads per core
SCALE = D ** -0.5
N_WARM = 30     # PE clock-ramp dummy transposes (~4.5us, spans the input DMA)
GAP_NS = 800    # filler budget per exp gap


def build_nc(NSEQ=2048):
    KT = NSEQ // 128    # key tiles
    QC = NSEQ // 512    # query chunks
    DKT = DIM // 128    # contraction chunks for projections
    NG = H * QC         # steps: gqc = qc*H + h
    NT = (KT + 2) // 3  # scores/exp tiles per step

    nc = bacc.Bacc("TRN2", target_bir_lowering=False, debug=False)
    # weights arrive pre-arranged by the host into their SBUF layout so each
    # DMA partition line is one long contiguous run (<512B runs pay 2x DMA);
    # wq is head-pair-major so the p=0 half can be DMA'd first
    xT = nc.dram_tensor("xt", [DIM, NSEQ], BF16, kind="ExternalInput").ap()
    wq = nc.dram_tensor("wq", [128, 2 * DKT * 128], BF16, kind="ExternalInput").ap()
    wkk = nc.dram_tensor("wkk", [128, DKT * 128], BF16, kind="ExternalInput").ap()
    wv = nc.dram_tensor("wv", [128, DKT * D], BF16, kind="ExternalInput").ap()
    wo = nc.dram_tensor("wo", [128, 2 * DIM], BF16, kind="ExternalInput").ap()
    out = nc.dram_tensor("out", [DIM, NSEQ], BF16, kind="ExternalOutput").ap()

    with tile.TileContext(nc) as tc, ExitStack() as ctx:
        sb = ctx.enter_context(tc.tile_pool(name="sb", bufs=1))

        wq_sb = sb.tile([128, 2, DKT, 128], BF16)
        wkk_sb = sb.tile([128, DKT, 128], BF16)
        wv_sb = sb.tile([128, DKT, D], BF16)
        wo_sb = sb.tile([128, 2, DIM], BF16)
        identb = sb.tile([128, 128], BF16)
        ones_k = sb.tile([128, 1], BF16)
        warm = sb.tile([128, 1], F32)

        xts = sb.tile([128, DKT, NSEQ], BF16)     # x^T, d-chunk t at [:, t, :]
        kkT = sb.tile([128, NSEQ], BF16)          # K^T duplicated rows 0-63 == 64-127
        qt = [sb.tile([128, NSEQ], BF16, name=f"qt{p}") for p in range(2)]
        vn = sb.tile([128, KT, D + 1], BF16)      # V seq-major + ones col 64
        # P^T ring, flat over 48 units of [128,512]; unit (gqc,kt) lives at
        # ring index u%48 = (gqc%3)*KT + kt, so consecutive units are
        # contiguous and every exp tile is a full 3 units
        ptr = sb.tile([128, 3 * KT * 512], BF16)
        aout = sb.tile([128, H, NSEQ // 128, D], BF16)
        aoutT = sb.tile([128, 2, NSEQ], BF16)

        make_identity(nc, identb)
        nc.vector.memset(ones_k, 1.0)
        nc.vector.memset(vn, 1.0)
        # preload the exp table off the critical path
        nc.scalar.activation(out=warm, in_=ones_k, func=EXPF, scale=1.0)

        # input DMA: first x chunk and the K/Q weights lead the queue (they
        # gate pipeline start); V/O weights ride the gpsimd queue.
        def dma_xchunk(qc):
            nc.sync.dma_start(
                out=xts[:, :, ds(qc * 512, 512)],
                in_=xT[:, ds(qc * 512, 512)].rearrange("(t p) m -> p t m", p=128),
            )

        # single serial DMA device: strict priority order on one queue; the
        # first x chunk is split so K projection starts on its first half
        nc.sync.dma_start(out=wkk_sb, in_=wkk.rearrange("p (t m) -> p t m", t=DKT))
        for dh in range(2):
            nc.sync.dma_start(
                out=xts[:, ds(dh * 4, 4), 0:512],
                in_=xT[ds(dh * 512, 512), 0:512].rearrange("(t p) m -> p t m", p=128),
            )
        nc.sync.dma_start(out=wq_sb[:, 0, :, :],
                          in_=wq[:, 0:DKT * 128].rearrange("p (t m) -> p t m", t=DKT))
        # later chunks in column halves: kkT key-tiles become available at the
        # pace the Act engine consumes them in step 0
        for qc in range(1, QC):
            for hh in range(2):
                c0 = qc * 512 + hh * 256
                nc.sync.dma_start(
                    out=xts[:, :, ds(c0, 256)],
                    in_=xT[:, ds(c0, 256)].rearrange("(t p) m -> p t m", p=128),
                )
        nc.sync.dma_start(out=wq_sb[:, 1, :, :],
                          in_=wq[:, DKT * 128:].rearrange("p (t m) -> p t m", t=DKT))
        nc.sync.dma_start(out=wv_sb, in_=wv.rearrange("p (t m) -> p t m", t=DKT))
        nc.sync.dma_start(out=wo_sb, in_=wo.rearrange("p (t m) -> p t m", t=2))

        scp = ctx.enter_context(tc.tile_pool(name="scp", bufs=2, space="PSUM"))
        psp = ctx.enter_context(tc.tile_pool(name="psp", bufs=2, space="PSUM"))
        rp = ctx.enter_context(tc.tile_pool(name="rp", bufs=2))
        stp = ctx.enter_context(tc.tile_pool(name="stp", bufs=4))

        # PE clock warm-up: dependency-free transposes keep the tensor engine
        # busy through the input DMA so real work starts at full clock.
        wps = psp.tile([128, 1024], BF16, tag="ps", name="warmps")
        for _ in range(N_WARM):
            nc.tensor.transpose(wps[:, 0:128], identb, identb)

        def emit_k(qc, half=None):
            c0, w = (qc * 512, 512) if half is None else (qc * 512 + half * 256, 256)
            ps = psp.tile([128, 512], F32, tag="ps", name=f"kps{qc}_{half}")
            for d in range(DKT):
                nc.tensor.matmul(ps[:, 0:w], wkk_sb[:, d, :], xts[:, d, ds(c0, w)],
                                 start=(d == 0), stop=(d == DKT - 1))
            nc.vector.tensor_copy(kkT[:, ds(c0, w)], ps[:, 0:w])

        def emit_q(p, qc):
            ps = psp.tile([128, 512], F32, tag="ps", name=f"qps{p}_{qc}")
            for d in range(DKT):
                nc.tensor.matmul(ps, wq_sb[:, p, d, :],
                                 xts[:, d, ds(qc * 512, 512)],
                                 start=(d == 0), stop=(d == DKT - 1))
            nc.vector.tensor_copy(qt[p][:, ds(qc * 512, 512)], ps)

        def emit_v(st):
            ps = psp.tile([128, 512], F32, tag="ps", name=f"vps{st}")
            for d in range(DKT):
                nc.tensor.matmul(ps[:, 0:D], xts[:, d, ds(st * 128, 128)],
                                 wv_sb[:, d, :],
                                 start=(d == 0), stop=(d == DKT - 1))
            nc.vector.tensor_copy(vn[:, st, 0:D], ps[:, 0:D])

        ustate = {"u": 0, "psc": None}

        def emit_unit(gqc, kt):
            """One 512-col scores matmul; exp flushes every 3 units."""
            u = ustate["u"]
            qc, h = divmod(gqc, H)
            p, hb = h // 2, (h % 2) * 64
            pos = u % 3
            if pos == 0:
                ustate["psc"] = scp.tile([128, 1536], F32, tag="sc", name=f"sc{u}")
            nc.tensor.matmul(ustate["psc"][:, ds(pos * 512, 512)],
                             kkT[ds(hb, 64), ts(kt, 128)],
                             qt[p][ds(hb, 64), ds(qc * 512, 512)],
                             start=True, stop=True)
            ustate["u"] = u + 1
            if pos == 2 or u == NG * KT - 1:
                ul = pos + 1
                ru0 = (u - pos) % (3 * KT)
                nc.scalar.activation(out=ptr[:, ds(ru0 * 512, ul * 512)],
                                     in_=ustate["psc"][:, 0:ul * 512],
                                     func=EXPF, scale=SCALE)
                drain()
                base = (NG - 1) * KT
                if u == base + 5:
                    pv_partial(NG - 1, 0, 6)
                elif u == base + 8:
                    pv_partial(NG - 1, 6, 9)
                elif u == base + 14:
                    pv_partial(NG - 1, 9, 15)

        # ---- filler queue: (est PE ns, emit_fn), drained between score tiles
        F = deque()

        def drain(budget=GAP_NS):
            spent = 0
            while F and spent < budget:
                cost, fn = F.popleft()
                fn()
                spent += cost

        pv_state = {}

        def pv_partial(gqc, lo, hi):
            """PV accumulation for all 4 query subtiles over key tiles [lo,hi)."""
            slot = gqc % 3
            if lo == 0:
                pv_state[gqc] = psp.tile([128, 512], F32, tag="ps", name=f"pv{gqc}")
            pv = pv_state[gqc]
            for j in range(4):
                for kt in range(lo, hi):
                    # start only on the tile's very first matmul: a start on
                    # j>0 would re-mark the whole PSUM bank pending-zero and
                    # wipe j0's partial sums on its next accumulate
                    nc.tensor.matmul(pv[:, ds(j * 65, 65)],
                                     ptr[:, ds((slot * KT + kt) * 512 + j * 128, 128)],
                                     vn[:, kt, :],
                                     start=(kt == 0 and j == 0), stop=(kt == KT - 1),
                                     skip_group_check=True)

        def pv_norm(gqc):
            qc, h = divmod(gqc, H)
            pv = pv_state.pop(gqc)
            pvs = rp.tile([128, 4 * (D + 1)], F32, tag="pvs", name=f"pvs{gqc}")
            nc.vector.tensor_copy(pvs, pv[:, 0:4 * (D + 1)])
            r = rp.tile([128, 4], F32, tag="r", name=f"r{gqc}")
            for j in range(4):
                nc.vector.reciprocal(out=r[:, ds(j, 1)],
                                     in_=pvs[:, ds(j * 65 + D, 1)])
            for j in range(4):
                nc.gpsimd.tensor_scalar_mul(aout[:, h, qc * 4 + j, :],
                                            pvs[:, ds(j * 65, D)], r[:, ds(j, 1)])

        def q_pv(gqc):
            """Queue PV j-blocks + normalization for step gqc."""

            def mk_j(j):
                def go():
                    slot = gqc % 3
                    if j == 0:
                        pv_state[gqc] = psp.tile([128, 512], F32, tag="ps",
                                                 name=f"pv{gqc}")
                    pv = pv_state[gqc]
                    for kt in range(KT):
                        nc.tensor.matmul(pv[:, ds(j * 65, 65)],
                                         ptr[:, ds((slot * KT + kt) * 512 + j * 128, 128)],
                                         vn[:, kt, :],
                                         start=(kt == 0 and j == 0), stop=(kt == KT - 1),
                                         skip_group_check=True)
                return go

            for j in range(4):
                F.append((433, mk_j(j)))
            F.append((0, lambda: pv_norm(gqc)))

        def q_tr_chunk(qc, c):
            """Transpose head-pair c (heads 2c, 2c+1) of qchunk qc into aoutT."""
            box = {}

            def mk_tr(hh):
                def go():
                    if hh % 2 == 0:
                        box["t"] = psp.tile([128, 1024], BF16, tag="ps",
                                            name=f"tr{qc}_{c}")
                    tr = box["t"]
                    for j in range(4):
                        nc.tensor.transpose(
                            tr[ds((hh % 2) * 64, 64), ds(j * 128, 128)],
                            aout[:, hh, qc * 4 + j, :], identb)
                return go

            def tr_copy():
                nc.vector.tensor_copy(aoutT[:, c, ds(qc * 512, 512)],
                                      box.pop("t")[:, 0:512])

            for hh in (2 * c, 2 * c + 1):
                F.append((213, mk_tr(hh)))
            F.append((0, tr_copy))

        def q_ods(qc, lo, hi, tail=False):
            """Output projection rows [128*lo, 128*hi); ods are processed in
            pairs sharing one staging tile and one DMA."""
            stt_box = {}

            def mk_od(od, use_scp, act_copy):
                def go():
                    if use_scp:
                        op = scp.tile([128, 3, 512], F32, tag="sc",
                                      name=f"op{qc}_{od}")[:, 0, :]
                    else:
                        op = psp.tile([128, 512], F32, tag="ps", name=f"op{qc}_{od}")
                    nc.tensor.matmul(op, wo_sb[:, 0, ts(od, 128)],
                                     aoutT[:, 0, ds(qc * 512, 512)],
                                     start=True, stop=False)
                    nc.tensor.matmul(op, wo_sb[:, 1, ts(od, 128)],
                                     aoutT[:, 1, ds(qc * 512, 512)],
                                     start=False, stop=True)
                    if od % 2 == 0:
                        stt_box[od] = stp.tile([128, 2, 512], BF16, tag="st",
                                               name=f"st{qc}_{od}")
                    stt = stt_box.get(od, stt_box.get(od - 1))
                    dst = stt[:, od % 2, :]
                    if act_copy:
                        nc.scalar.activation(out=dst, in_=op,
                                             func=mybir.ActivationFunctionType.Copy,
                                             scale=1.0)
                    else:
                        nc.vector.tensor_copy(dst, op)
                    if od % 2 == 1:
                        stt2 = stt_box.pop(od - 1)
                        eng = nc.gpsimd if (tail and (od // 2) % 2 == 1) else nc.sync
                        eng.dma_start(
                            out=out[ds((od - 1) * 128, 256), ds(qc * 512, 512)]
                            .rearrange("(c p) m -> p c m", p=128),
                            in_=stt2)
                return go

            for od in range(lo, hi):
                F.append((427, mk_od(od, tail and od % 2 == 0, tail and od % 2 == 1)))

        # ---- step 0: fixed interleave to bootstrap K/Q ----
        emit_k(0)
        emit_q(0, 0)
        for kt in range(KT):
            emit_unit(0, kt)
            if kt in (2, 5, 8):
                emit_k(kt // 3 + 1, 0)
                emit_k(kt // 3 + 1, 1)
        emit_q(1, 0)

        # ---- main loop ----
        # per-loop filler plan (each transpose group lands >=1 loop after the
        # normalization it reads, so PE never stalls on the norm chain):
        #   ph==3: tr-chunk0(qc4) + Q(0, qc4+1)
        #   ph==0: Q(1, qc4)
        #   ph==1: tr-chunk1(qc4-1) + output rows 0-3 of qc4-1
        #   ph==2: output rows 4-7 of qc4-1
        for gqc in range(1, NG):
            if gqc == 1:
                for st in range(KT):
                    F.append((213, (lambda s: lambda: emit_v(s))(st)))
            q_pv(gqc - 1)
            qc4, ph = divmod(gqc, 4)
            if ph == 3:
                q_tr_chunk(qc4, 0)
                if qc4 + 1 < QC:
                    F.append((1707, (lambda q: lambda: emit_q(0, q))(qc4 + 1)))
            elif ph == 0:
                F.append((1707, (lambda q: lambda: emit_q(1, q))(qc4)))
            elif ph == 1 and qc4 >= 1:
                q_tr_chunk(qc4 - 1, 1)
                q_ods(qc4 - 1, 0, 4)
            elif ph == 2 and qc4 >= 1:
                q_ods(qc4 - 1, 4, 8)
            for kt in range(KT):
                emit_unit(gqc, kt)
        # ---- tail: j-granular PV residue -> norm -> transpose chains so the
        # last step's latencies overlap; then the final output projection
        gq, qc = NG - 1, QC - 1
        slot = gq % 3
        trt = psp.tile([128, 1024], BF16, tag="ps", name="tr_tail")
        for j in range(4):
            nc.tensor.transpose(trt[ds(0, 64), ds(j * 128, 128)],
                                aout[:, 2, qc * 4 + j, :], identb)
        pv = pv_state.pop(gq)
        r = rp.tile([128, 4], F32, tag="r", name="r_tail")
        for j in range(4):
            nc.tensor.matmul(pv[:, ds(j * 65, 65)],
                             ptr[:, ds((slot * KT + KT - 1) * 512 + j * 128, 128)],
                             vn[:, KT - 1, :],
                             start=False, stop=True,
                             skip_group_check=True)
            nc.vector.reciprocal(out=r[:, ds(j, 1)], in_=pv[:, ds(j * 65 + D, 1)])
            nc.vector.tensor_scalar_mul(aout[:, 3, qc * 4 + j, :],
                                        pv[:, ds(j * 65, D)], r[:, ds(j, 1)])
        for j in range(4):
            nc.tensor.transpose(trt[ds(64, 64), ds(j * 128, 128)],
                                aout[:, 3, qc * 4 + j, :], identb)
        nc.vector.tensor_copy(aoutT[:, 1, ds(qc * 512, 512)], trt[:, 0:512])
        q_ods(qc, 0, 8, tail=True)
        drain(budget=1 << 30)

    nc.compile()
    return nc


_CACHE = {}


def _get_nc(NSEQ):
    if NSEQ not in _CACHE:
        _CACHE[NSEQ] = build_nc(NSEQ)
    return _CACHE[NSEQ]


def kernel(x, Wq, Wk, Wv, Wo, bo):
    """Full-input entry point: shard over 8 cores, run, gather."""
    import ml_dtypes

    bf16 = ml_dtypes.bfloat16
    x, Wq, Wk, Wv, Wo, bo = (np.asarray(a, np.float32) for a in (x, Wq, Wk, Wv, Wo, bo))
    B, N, C = x.shape
    nc = _get_nc(N)
    xT_b = [np.ascontiguousarray(x[b].T).astype(bf16) for b in range(B)]

    def arr(w):
        """Pre-arrange [128*t, m] weight to SBUF layout [128, t*m]."""
        t = w.shape[0] // 128
        return np.ascontiguousarray(
            w.reshape(t, 128, -1).transpose(1, 0, 2).reshape(128, -1)).astype(bf16)

    def arr_q(w):
        """Pre-arrange Wq [128*t, 2*128] to head-pair-major [128, pp*t*m]."""
        t = w.shape[0] // 128
        return np.ascontiguousarray(
            w.reshape(t, 128, 2, 128).transpose(1, 2, 0, 3).reshape(128, -1)).astype(bf16)

    in_maps = []
    for c in range(8):
        b, g = c // 4, c % 4
        wk_g = Wk[:, g * D:(g + 1) * D]
        in_maps.append({
            "xt": xT_b[b],
            "wq": arr_q(Wq[:, g * 256:(g + 1) * 256]),
            "wkk": arr(np.concatenate([wk_g, wk_g], axis=1)),
            "wv": arr(Wv[:, g * D:(g + 1) * D]),
            "wo": arr(Wo[g * 256:(g + 1) * 256, :]),
        })
    res = bass_utils.run_bass_kernel_spmd(nc, in_maps, core_ids=list(range(8)))
    outs = [res.results[c]["out"] for c in range(8)]
    full = np.empty((B, N, C), np.float32)
    for b in range(B):
        acc = outs[4 * b].astype(np.float32)
        for g in range(1, 4):
            acc = acc + outs[4 * b + g].astype(np.float32)
        full[b] = acc.T + bo[None, :]
    return full
